# revision 23
# baseline (speedup 1.0000x reference)
"""Trainium2 Bass kernel for the dense CNN (conv1+pool -> untied/pointwise/gated mix -> pool -> fc).

Self-contained: hardcodes shapes for B=2048, 8-core data-parallel sharding.
kernel(**inputs) takes FULL inputs, returns (output [2048,10], flat [2048,800]).

Design (per core, 256 samples):
  stage 1  conv1 (1->16ch 5x5 pad2) as Toeplitz row-band matmuls: K=128 (4 y-shifted
           replicas x 32 padded cols) + K=33 bias pass; M=112 packs 8 channels x 14
           pooled-x columns, with even/odd conv-x in separate PSUM tiles so the 2x2
           maxpool is one free-dim tensor_reduce per parity + one fused max/relu.
  stage 2  SBUF->SBUF DMA shuffle into h_shift[(kx,c), px, y', b]: 5 x-shifted
           replicas of pooled h enabling zero-copy patch streaming (row 80 = ones).
  stage 3  untied + pointwise + gate convs fused: per output position, 5 accumulating
           matmuls (K=81) with stationary [d=wu-wpc | wpc | wl] (M=65, bias via ones
           row); PE-transpose to batch-major; sigmoid-gated mix at 128 partitions;
           2x2 maxpool fused into strided writes of flat.
  stage 4  fc: 7 accumulating matmuls over K=800 (+ones bias row), transpose, DMA out.
"""

import numpy as np
import ml_dtypes
from contextlib import ExitStack

import concourse.bass as bass
import concourse.tile as tile
from concourse import bacc
from concourse import mybir
from concourse.bass_utils import run_bass_kernel_spmd

F32 = mybir.dt.float32
BF16 = mybir.dt.bfloat16
NPBF16 = ml_dtypes.bfloat16

N_CORES = 8
B = 2048
B0 = B // N_CORES          # 256 samples per core
BH = B0 // 2               # 128, batch half (partition dim for transposed stages)

# quad-major position ordering for the 10x10 untied-conv grid (2x2 pool groups)
QUAD_ORDER = [(2 * qy + dy, 2 * qx + dx)
              for qy in range(5) for qx in range(5)
              for dy in range(2) for dx in range(2)]

SCHUNK = 10   # untied stationary streaming chunk (positions per DMA)


def build_program():
    nc = bacc.Bacc("TRN2", target_bir_lowering=False, debug=False, num_devices=N_CORES)
    I = {}
    for name, shape, dt in [
        ("xb4", [128, 28, B0], BF16),
        ("xb1", [33, 28, B0], BF16),
        ("sa", [2, 2, 128, 112], BF16),     # [cp, parity, K, M]
        ("sb", [2, 2, 33, 112], BF16),
        ("su", [81, 100, 5, 65], BF16),
        ("fcwt", [128, 7, 10], BF16),
        ("identb", [128, 128], BF16),
        ("identf", [128, 128], F32),
    ]:
        I[name] = nc.declare_dram_parameter(name, shape, dt, isOutput=False)
    O = {
        "flat": nc.declare_dram_parameter("flat", [B0, 800], F32, isOutput=True),
        "out": nc.declare_dram_parameter("out", [B0, 10], F32, isOutput=True),
    }

    with tile.TileContext(nc) as tc:
        with ExitStack() as ctx:
            _build(ctx, tc, I, O)
    nc.compile()
    return nc


def _build(ctx, tc, I, O):
    nc = tc.nc
    AF = mybir.ActivationFunctionType
    ALU = mybir.AluOpType
    AX = mybir.AxisListType

    consts = ctx.enter_context(tc.tile_pool(name="consts", bufs=1))
    spool = ctx.enter_context(tc.tile_pool(name="sstream", bufs=3))
    qpool = ctx.enter_context(tc.tile_pool(name="q", bufs=3))
    h2pool = ctx.enter_context(tc.tile_pool(name="h2", bufs=1))
    hspool = ctx.enter_context(tc.tile_pool(name="hshift", bufs=1))
    usbpool = ctx.enter_context(tc.tile_pool(name="usb", bufs=4))
    sqpool = ctx.enter_context(tc.tile_pool(name="sq", bufs=50))
    mixpool = ctx.enter_context(tc.tile_pool(name="mix", bufs=4))
    flatpool = ctx.enter_context(tc.tile_pool(name="flatT", bufs=1))
    fcpool = ctx.enter_context(tc.tile_pool(name="fcmisc", bufs=1))

    ps_c1 = ctx.enter_context(tc.tile_pool(name="ps_c1", bufs=4, space="PSUM"))
    ps_u = ctx.enter_context(tc.tile_pool(name="ps_u", bufs=2, space="PSUM"))
    ps_q = ctx.enter_context(tc.tile_pool(name="ps_q", bufs=2, space="PSUM"))

    # ---- load persistent constants ----
    xb4 = consts.tile([128, 28, B0], BF16)
    nc.sync.dma_start(xb4[:], I["xb4"][:])
    xb1 = consts.tile([33, 28, B0], BF16)
    nc.sync.dma_start(xb1[:], I["xb1"][:])
    sa_t = [[consts.tile([128, 112], BF16, tag=f"sa{cp}{par}", name=f"sa{cp}{par}")
             for par in range(2)] for cp in range(2)]
    sb_t = [[consts.tile([33, 112], BF16, tag=f"sb{cp}{par}", name=f"sb{cp}{par}")
             for par in range(2)] for cp in range(2)]
    for cp in range(2):
        for par in range(2):
            nc.sync.dma_start(sa_t[cp][par][:], I["sa"][cp, par])
            nc.sync.dma_start(sb_t[cp][par][:], I["sb"][cp, par])
    fcwt = consts.tile([128, 7, 10], BF16, tag="fcwt")
    nc.sync.dma_start(fcwt[:], I["fcwt"][:])
    identb = consts.tile([128, 128], BF16, tag="identb")
    nc.sync.dma_start(identb[:], I["identb"][:])
    identf = consts.tile([128, 128], F32, tag="identf")
    nc.sync.dma_start(identf[:], I["identf"][:])

    # h_shift[(kx*16+c), px, y', b] = pooled_h[c, y', px+kx, b];  row 80 = ones (bias row)
    h_shift = hspool.tile([81, 10, 14, B0], BF16)
    nc.gpsimd.memset(h_shift[64:81, :, :, :], 1.0)  # rows 64..79 overwritten by shuffle
    # pool-proc observation token (same Pool sem, later tick than the memset above)
    scr_pool = consts.tile([1, 16], BF16, tag="scr_pool")
    nc.gpsimd.memset(scr_pool[:], 0.0)

    # prefetch first two untied-stationary chunks early (their DMA lanes get
    # re-observed via the lane ladder below, keeping matmul wait counts <= 2)
    def load_chunk(ci):
        t = spool.tile([81, SCHUNK, 5, 65], BF16, tag="su", name=f"su{ci}")
        nc.sync.dma_start(t[:], I["su"][:, ci * SCHUNK:(ci + 1) * SCHUNK, :, :])
        return t
    s_chunks = {0: load_chunk(0), 1: load_chunk(1)}

    # ---- stage 1: conv1 (Toeplitz row-band) + relu + 2x2 maxpool ----
    # M layout: col = member*56 + co_l*14 + oxh;  channel c = cp*8 + member*4 + co_l
    h2_t = []
    for cp in range(2):
        h2 = h2pool.tile([112, 14, B0], BF16, tag=f"h2_{cp}", name=f"h2_{cp}")
        h2_t.append(h2)
        for yp in range(14):
            y = yp * 2
            pse = ps_c1.tile([112, 2, B0], F32, tag="psc1", name=f"pse{cp}_{yp}")
            nc.tensor.matmul(pse[:], sa_t[cp][0][:], xb4[:, y:y + 2, :],
                             start=True, stop=False)
            nc.tensor.matmul(pse[:], sb_t[cp][0][:], xb1[:, y:y + 2, :],
                             start=False, stop=True)
            pso = ps_c1.tile([112, 2, B0], F32, tag="psc1", name=f"pso{cp}_{yp}")
            nc.tensor.matmul(pso[:], sa_t[cp][1][:], xb4[:, y:y + 2, :],
                             start=True, stop=False)
            nc.tensor.matmul(pso[:], sb_t[cp][1][:], xb1[:, y:y + 2, :],
                             start=False, stop=True)
            # maxpool over conv-y pair (innermost after free permute), per parity
            q1 = qpool.tile([112, B0], BF16, tag="q1")
            nc.vector.tensor_reduce(q1[:], pse[:].rearrange("p a b -> p b a"),
                                    axis=AX.X, op=ALU.max)
            q2 = qpool.tile([112, B0], BF16, tag="q2")
            nc.vector.tensor_reduce(q2[:], pso[:].rearrange("p a b -> p b a"),
                                    axis=AX.X, op=ALU.max)
            # maxpool over conv-x parity + relu: h2 = max(max(q1, 0), q2)
            nc.vector.scalar_tensor_tensor(h2[:, yp, :], q1[:], 0.0, q2[:],
                                           op0=ALU.max, op1=ALU.max)

    # ---- stage 2: shuffle h2 -> h_shift (SBUF->SBUF DMA, 80 descriptors) ----
    for kx in range(5):
        for c in range(16):
            cp, member, co_l = c // 8, (c % 8) // 4, c % 4
            pb = member * 56 + co_l * 14 + kx
            src = h2_t[cp][pb: pb + 10, :, :]                        # [10px, 14, B0]
            dst = h_shift[c + kx * 16: c + kx * 16 + 1, :, :, :]     # [1, 10px, 14, B0]
            nc.sync.dma_start(dst, src)

    # Lane-observation ladder: walrus caps compute instructions at 2 sem waits,
    # but the first matmul reading h_shift would need waits on every HW-DMA
    # lane the 80-descriptor shuffle used.  Instead: barrier orders the SP DMA
    # stream; 8 trailing 1-line DMAs land on all 8 round-robin lanes (queue
    # FIFO => their completion implies the shuffle's); dummy ldweights make the
    # PE observe each lane (and the Pool memset) one wait at a time.
    tc.strict_bb_all_engine_barrier()
    scr_lane = [consts.tile([1, 16], BF16, tag=f"scrl{k}", name=f"scrl{k}")
                for k in range(8)]
    for k in range(8):
        nc.sync.dma_start(scr_lane[k][:], I["identb"][0:1, 0:16])
    for k in range(8):
        nc.tensor.ldweights(scr_lane[k][:])
    nc.tensor.ldweights(scr_pool[:])
    tc.strict_bb_all_engine_barrier()

    # ---- stage 3: untied + pointwise + gate convs; transpose; gated mix; pool ----
    flatT = [flatpool.tile([BH, 800], F32, tag=f"flatT{h}", name=f"flatT{h}") for h in range(2)]

    for quad in range(25):
        pprime = (quad // 5) * 5 + (quad % 5)   # pooled position index py'*5+px'
        psq = [ps_q.tile([128, 4, 66], F32, tag="psq", name=f"psq{quad}_{i}") for i in range(2)]
        for qpos in range(4):
            qi = quad * 4 + qpos
            ci = qi // SCHUNK
            if qi % SCHUNK == 0 and ci + 1 not in s_chunks and ci + 1 < 10:
                s_chunks[ci + 1] = load_chunk(ci + 1)
            s_t = s_chunks[ci]
            py, px = QUAD_ORDER[qi]
            psu = ps_u.tile([65, B0], F32)
            for ky in range(5):
                nc.tensor.matmul(psu[:], s_t[:, qi % SCHUNK, ky, :],
                                 h_shift[:, px, py + ky, :],
                                 start=(ky == 0), stop=(ky == 4))
            usb = usbpool.tile([65, B0], F32)
            nc.scalar.copy(usb[:], psu[:])
            for h in range(2):
                nc.tensor.transpose(psq[h][:, qpos, 0:65], usb[:, h * BH:(h + 1) * BH],
                                    identf[0:65, 0:65])
        for h in range(2):
            gsrc = sqpool.tile([128, 4, 1], F32, tag="gsrc", name=f"gsrc{quad}_{h}")
            nc.vector.tensor_copy(gsrc[:], psq[h][:, :, 64:65])
            sq = sqpool.tile([128, 4, 1], F32, tag="sq", name=f"sq{quad}_{h}")
            nc.scalar.activation(sq[:], gsrc[:], AF.Sigmoid)
            tmpq = mixpool.tile([128, 4, 32], F32, tag="tmpq")
            nc.vector.tensor_tensor(tmpq[:], psq[h][:, :, 0:32],
                                    sq[:].broadcast_to((128, 4, 32)), op=ALU.mult)
            mixq = mixpool.tile([128, 4, 32], F32, tag="mixq")
            nc.vector.tensor_tensor(mixq[:], tmpq[:], psq[h][:, :, 32:64], op=ALU.add)
            t1 = mixpool.tile([128, 2, 32], F32, tag="t1")
            nc.vector.tensor_tensor(t1[:], mixq[:, 0::2, :], mixq[:, 1::2, :], op=ALU.max)
            nc.vector.scalar_tensor_tensor(
                flatT[h][:, pprime::25], t1[:, 0, :], 0.0, t1[:, 1, :],
                op0=ALU.max, op1=ALU.max)

    # ---- stage 4: flat output DMA + re-transpose to K-major + fc ----
    for h in range(2):
        nc.sync.dma_start(O["flat"][h * BH:(h + 1) * BH, :], flatT[h][:])

    flatC = fcpool.tile([128, 7, B0], BF16, tag="flatC")
    nc.gpsimd.memset(flatC[32:64, 6, :], 1.0)  # row 32 = fc bias ones; 33+ never read (K=33)
    for j in range(7):
        w = 128 if j < 6 else 32
        for h in range(2):
            pst = ps_q.tile([128, 4, 66], F32, tag="psq", name=f"pst{j}_{h}")
            pt = pst[:].rearrange("p a b -> p (a b)")[0:w, 0:BH]
            nc.tensor.transpose(pt, flatT[h][:, j * 128:j * 128 + w],
                                identf[0:BH, 0:BH])
            nc.scalar.copy(flatC[0:w, j, h * BH:(h + 1) * BH], pt)

    psfc = ps_u.tile([65, B0], F32, tag="psu")
    for j in range(7):
        k = 128 if j < 6 else 33
        nc.tensor.matmul(psfc[0:10, :], fcwt[0:k, j, :], flatC[0:k, j, :],
                         start=(j == 0), stop=(j == 6))
    osb = fcpool.tile([10, B0], F32, tag="osb")
    nc.scalar.copy(osb[:], psfc[0:10, :])
    for h in range(2):
        pso2 = ps_q.tile([128, 4, 66], F32, tag="psq", name=f"pso2_{h}")
        po = pso2[:].rearrange("p a b -> p (a b)")[0:BH, 0:10]
        nc.tensor.transpose(po, osb[:, h * BH:(h + 1) * BH], identf[0:10, 0:10])
        ot = fcpool.tile([BH, 10], F32, tag=f"ot{h}", name=f"ot{h}")
        nc.scalar.copy(ot[:], po)
        nc.sync.dma_start(O["out"][h * BH:(h + 1) * BH, :], ot[:])


# ---------------------------------------------------------------------------
# host-side data prep
# ---------------------------------------------------------------------------

def _prep_weights(conv1_w, conv1_b, uc_w, uc_b, pc_w, pc_b, wl_w, wl_b, fc_w, fc_b):
    w1 = conv1_w[:, 0]  # [16, 5, 5]
    # M col layout: member*56 + co_l*14 + oxh; parity = ox % 2 selects the psum tile
    SA = np.zeros((2, 2, 128, 112), np.float32)
    SB = np.zeros((2, 2, 33, 112), np.float32)
    oxh = np.arange(14)
    for cp in range(2):
        for member in range(2):
            for co_l in range(4):
                co = cp * 8 + member * 4 + co_l
                col = member * 56 + co_l * 14 + oxh
                for par in range(2):
                    ox = 2 * oxh + par
                    for kx in range(5):
                        for g in range(4):
                            SA[cp, par, g * 32 + ox + kx, col] = w1[co, g, kx]
                        SB[cp, par, ox + kx, col] = w1[co, 4, kx]
                    SB[cp, par, 32, col] = conv1_b[co]

    # untied stationary: [81, 100(quad order), 5ky, 65]
    wu = uc_w.reshape(100, 16, 5, 5, 32)          # [p, c, ky, kx, o]
    wpc = pc_w.transpose(1, 2, 3, 0)               # [c, ky, kx, o]
    wwl = wl_w[0]                                  # [c, ky, kx]
    SU = np.zeros((81, 100, 5, 65), np.float32)
    for qi, (py, px) in enumerate(QUAD_ORDER):
        p = py * 10 + px
        for kx in range(5):
            r = slice(kx * 16, kx * 16 + 16)
            # SU[kx*16+c, qi, ky, :] — slices below are [c, ky, o] / [c, ky]
            SU[r, qi, :, 0:32] = wu[p, :, :, kx, :] - wpc[:, :, kx, :]
            SU[r, qi, :, 32:64] = wpc[:, :, kx, :]
            SU[r, qi, :, 64] = wwl[:, :, kx]
        SU[80, qi, 0, 0:32] = uc_b[0, :, py, px] - pc_b
        SU[80, qi, 0, 32:64] = pc_b
        SU[80, qi, 0, 64] = wl_b[0]

    fcwt = np.zeros((128, 7, 10), np.float32)
    for j in range(6):
        fcwt[:, j, :] = fc_w[:, j * 128:(j + 1) * 128].T
    fcwt[0:32, 6, :] = fc_w[:, 768:800].T
    fcwt[32, 6, :] = fc_b
    return (SA.astype(NPBF16), SB.astype(NPBF16), SU.astype(NPBF16),
            fcwt.astype(NPBF16))


def _prep_x(xs):
    """xs: [B0, 28, 28] f32 -> xb4 [128, 28, B0], xb1 [33, 28, B0] (bf16)."""
    xt = np.ascontiguousarray(xs.transpose(2, 1, 0))  # [xcol, yrow, b]
    xb4 = np.zeros((4, 32, 28, B0), np.float32)
    for g in range(4):
        ylo, yhi = max(0, 2 - g), min(28, 30 - g)
        xb4[g, 2:30, ylo:yhi, :] = xt[:, ylo + g - 2:yhi + g - 2, :]
    xb1 = np.zeros((33, 28, B0), np.float32)
    xb1[2:30, 0:26, :] = xt[:, 2:28, :]
    xb1[32] = 1.0
    return xb4.reshape(128, 28, B0).astype(NPBF16), xb1.astype(NPBF16)


_CACHED_NC = None


def _get_nc():
    global _CACHED_NC
    if _CACHED_NC is None:
        _CACHED_NC = build_program()
    return _CACHED_NC


def run(inputs, trace=False, **kw):
    x = np.asarray(inputs["x"], np.float32)
    SA, SB, SU, fcwt = _prep_weights(
        np.asarray(inputs["conv1_w"], np.float32), np.asarray(inputs["conv1_b"], np.float32),
        np.asarray(inputs["uc_w"], np.float32), np.asarray(inputs["uc_b"], np.float32),
        np.asarray(inputs["pc_w"], np.float32), np.asarray(inputs["pc_b"], np.float32),
        np.asarray(inputs["wl_w"], np.float32), np.asarray(inputs["wl_b"], np.float32),
        np.asarray(inputs["fc_w"], np.float32), np.asarray(inputs["fc_b"], np.float32))
    identb = np.eye(128, dtype=NPBF16)
    identf = np.eye(128, dtype=np.float32)

    in_maps = []
    for c in range(N_CORES):
        xb4, xb1 = _prep_x(x[c * B0:(c + 1) * B0, 0])
        in_maps.append({
            "xb4": xb4, "xb1": xb1, "sa": SA, "sb": SB, "su": SU,
            "fcwt": fcwt, "identb": identb, "identf": identf,
        })

    nc = _get_nc()
    res = run_bass_kernel_spmd(nc, in_maps, list(range(N_CORES)), trace=trace, **kw)
    flat = np.concatenate([res.results[c]["flat"] for c in range(N_CORES)], axis=0)
    out = np.concatenate([res.results[c]["out"] for c in range(N_CORES)], axis=0)
    return (out.astype(np.float32), flat.astype(np.float32)), res


def kernel(**inputs):
    (out, flat), _ = run(inputs)
    return (out, flat)


# revision 25
# speedup vs baseline: 1.2168x; 1.2168x over previous
"""Trainium2 Bass kernel for the dense CNN (conv1+pool -> untied/pointwise/gated mix -> pool -> fc).

Self-contained: hardcodes shapes for B=2048, 8-core data-parallel sharding.
kernel(**inputs) takes FULL inputs, returns (output [2048,10], flat [2048,800]).

Design (per core, 256 samples):
  stage 1  conv1 (1->16ch 5x5 pad2) as Toeplitz row-band matmuls: K=128 (4 y-shifted
           replicas x 32 padded cols) + K=33 bias pass; M=112 packs 8 channels x 14
           pooled-x columns, with even/odd conv-x in separate PSUM tiles so the 2x2
           maxpool is one free-dim tensor_reduce per parity + one fused max/relu.
  stage 2  SBUF->SBUF DMA shuffle into h_shift[(kx,c), px, y', b]: 5 x-shifted
           replicas of pooled h enabling zero-copy patch streaming (row 80 = ones).
  stage 3  untied + pointwise + gate convs fused: per output position, 5 accumulating
           matmuls (K=81) with stationary [d=wu-wpc | wpc | wl] (M=65, bias via ones
           row); PE-transpose to batch-major; sigmoid-gated mix at 128 partitions;
           2x2 maxpool fused into strided writes of flat.
  stage 4  fc: 7 accumulating matmuls over K=800 (+ones bias row), transpose, DMA out.
"""

import numpy as np
import ml_dtypes
from contextlib import ExitStack

import concourse.bass as bass
import concourse.tile as tile
from concourse import bacc
from concourse import mybir
from concourse.bass_utils import run_bass_kernel_spmd

F32 = mybir.dt.float32
BF16 = mybir.dt.bfloat16
NPBF16 = ml_dtypes.bfloat16

N_CORES = 8
B = 2048
B0 = B // N_CORES          # 256 samples per core
BH = B0 // 2               # 128, batch half (partition dim for transposed stages)

# quad-major position ordering for the 10x10 untied-conv grid (2x2 pool groups)
QUAD_ORDER = [(2 * qy + dy, 2 * qx + dx)
              for qy in range(5) for qx in range(5)
              for dy in range(2) for dx in range(2)]

SCHUNK = 20   # untied stationary streaming chunk (positions per DMA)
NCHUNKS = 100 // SCHUNK


def build_program():
    nc = bacc.Bacc("TRN2", target_bir_lowering=False, debug=False, num_devices=N_CORES)
    I = {}
    for name, shape, dt in [
        ("xb4", [128, 28, B0], BF16),
        ("xb1", [33, 28, B0], BF16),
        ("sa", [2, 2, 128, 112], BF16),     # [cp, parity, K, M]
        ("sb", [2, 2, 33, 112], BF16),
        ("su", [81, 100, 5, 65], BF16),
        ("fcwt", [128, 7, 10], BF16),
        ("identb", [128, 128], BF16),
        ("identf", [128, 128], F32),
    ]:
        I[name] = nc.declare_dram_parameter(name, shape, dt, isOutput=False)
    O = {
        "flat": nc.declare_dram_parameter("flat", [B0, 800], F32, isOutput=True),
        "out": nc.declare_dram_parameter("out", [B0, 10], F32, isOutput=True),
    }

    with tile.TileContext(nc) as tc:
        with ExitStack() as ctx:
            _build(ctx, tc, I, O)
    nc.compile()
    return nc


def _build(ctx, tc, I, O):
    nc = tc.nc
    AF = mybir.ActivationFunctionType
    ALU = mybir.AluOpType
    AX = mybir.AxisListType

    consts = ctx.enter_context(tc.tile_pool(name="consts", bufs=1))
    spool = ctx.enter_context(tc.tile_pool(name="sstream", bufs=3))
    qpool = ctx.enter_context(tc.tile_pool(name="q", bufs=3))
    h2pool = ctx.enter_context(tc.tile_pool(name="h2", bufs=1))
    hspool = ctx.enter_context(tc.tile_pool(name="hshift", bufs=1))
    sqpool = ctx.enter_context(tc.tile_pool(name="sq", bufs=50))
    mixpool = ctx.enter_context(tc.tile_pool(name="mix", bufs=4))
    flatpool = ctx.enter_context(tc.tile_pool(name="flatT", bufs=1))
    fcpool = ctx.enter_context(tc.tile_pool(name="fcmisc", bufs=1))

    ps_c1 = ctx.enter_context(tc.tile_pool(name="ps_c1", bufs=4, space="PSUM"))
    ps_q = ctx.enter_context(tc.tile_pool(name="ps_q", bufs=4, space="PSUM"))

    # ---- load persistent constants ----
    xb4 = consts.tile([128, 28, B0], BF16)
    nc.sync.dma_start(xb4[:], I["xb4"][:])
    xb1 = consts.tile([33, 28, B0], BF16)
    nc.sync.dma_start(xb1[:], I["xb1"][:])
    sa_t = [[consts.tile([128, 112], BF16, tag=f"sa{cp}{par}", name=f"sa{cp}{par}")
             for par in range(2)] for cp in range(2)]
    sb_t = [[consts.tile([33, 112], BF16, tag=f"sb{cp}{par}", name=f"sb{cp}{par}")
             for par in range(2)] for cp in range(2)]
    for cp in range(2):
        for par in range(2):
            nc.sync.dma_start(sa_t[cp][par][:], I["sa"][cp, par])
            nc.sync.dma_start(sb_t[cp][par][:], I["sb"][cp, par])
    fcwt = consts.tile([128, 7, 10], BF16, tag="fcwt")
    nc.sync.dma_start(fcwt[:], I["fcwt"][:])
    identf = consts.tile([128, 128], F32, tag="identf")
    nc.sync.dma_start(identf[:], I["identf"][:])

    # h_shift[(kx*16+c), px, y', b] = pooled_h[c, y', px+kx, b];  row 80 = ones (bias row)
    h_shift = hspool.tile([81, 10, 14, B0], BF16)
    nc.gpsimd.memset(h_shift[64:81, :, :, :], 1.0)  # rows 64..79 overwritten by shuffle
    # pool-proc observation token (same Pool sem, later tick than the memset above)
    scr_pool = consts.tile([1, 16], BF16, tag="scr_pool")
    nc.gpsimd.memset(scr_pool[:], 0.0)

    # prefetch first two untied-stationary chunks early (their DMA lanes get
    # re-observed via the lane ladder below, keeping matmul wait counts <= 2)
    def load_chunk(ci):
        t = spool.tile([81, SCHUNK, 5, 65], BF16, tag="su", name=f"su{ci}")
        nc.sync.dma_start(t[:], I["su"][:, ci * SCHUNK:(ci + 1) * SCHUNK, :, :])
        return t
    s_chunks = {0: load_chunk(0), 1: load_chunk(1)}

    # ---- stage 1: conv1 (Toeplitz row-band) + relu + 2x2 maxpool ----
    # M layout: col = member*56 + co_l*14 + oxh;  channel c = cp*8 + member*4 + co_l
    h2_t = []
    for cp in range(2):
        h2 = h2pool.tile([112, 14, B0], BF16, tag=f"h2_{cp}", name=f"h2_{cp}")
        h2_t.append(h2)
        for yp in range(14):
            y = yp * 2
            pse = ps_c1.tile([112, 2, B0], F32, tag="psc1", name=f"pse{cp}_{yp}")
            nc.tensor.matmul(pse[:], sa_t[cp][0][:], xb4[:, y:y + 2, :],
                             start=True, stop=False)
            nc.tensor.matmul(pse[:], sb_t[cp][0][:], xb1[:, y:y + 2, :],
                             start=False, stop=True)
            pso = ps_c1.tile([112, 2, B0], F32, tag="psc1", name=f"pso{cp}_{yp}")
            nc.tensor.matmul(pso[:], sa_t[cp][1][:], xb4[:, y:y + 2, :],
                             start=True, stop=False)
            nc.tensor.matmul(pso[:], sb_t[cp][1][:], xb1[:, y:y + 2, :],
                             start=False, stop=True)
            # maxpool over conv-y pair (innermost after free permute), per parity
            q1 = qpool.tile([112, B0], BF16, tag="q1")
            nc.vector.tensor_reduce(q1[:], pse[:].rearrange("p a b -> p b a"),
                                    axis=AX.X, op=ALU.max)
            q2 = qpool.tile([112, B0], BF16, tag="q2")
            nc.vector.tensor_reduce(q2[:], pso[:].rearrange("p a b -> p b a"),
                                    axis=AX.X, op=ALU.max)
            # maxpool over conv-x parity + relu: h2 = max(max(q1, 0), q2)
            nc.vector.scalar_tensor_tensor(h2[:, yp, :], q1[:], 0.0, q2[:],
                                           op0=ALU.max, op1=ALU.max)

    # ---- stage 2: shuffle h2 -> h_shift (SBUF->SBUF DMA, 80 descriptors) ----
    for kx in range(5):
        for c in range(16):
            cp, member, co_l = c // 8, (c % 8) // 4, c % 4
            pb = member * 56 + co_l * 14 + kx
            src = h2_t[cp][pb: pb + 10, :, :]                        # [10px, 14, B0]
            dst = h_shift[c + kx * 16: c + kx * 16 + 1, :, :, :]     # [1, 10px, 14, B0]
            nc.sync.dma_start(dst, src)

    # Lane-observation ladder: walrus caps compute instructions at 2 sem waits,
    # but the first matmul reading h_shift would need waits on every HW-DMA
    # lane the 80-descriptor shuffle used.  Instead: barrier orders the SP DMA
    # stream; 8 trailing 1-line DMAs land on all 8 round-robin lanes (queue
    # FIFO => their completion implies the shuffle's); dummy ldweights make the
    # PE observe each lane (and the Pool memset) one wait at a time.
    tc.strict_bb_all_engine_barrier()
    scr_lane = [consts.tile([1, 16], BF16, tag=f"scrl{k}", name=f"scrl{k}")
                for k in range(8)]
    for k in range(8):
        nc.sync.dma_start(scr_lane[k][:], I["identb"][0:1, 0:16])
    for k in range(8):
        nc.tensor.ldweights(scr_lane[k][:])
    nc.tensor.ldweights(scr_pool[:])
    tc.strict_bb_all_engine_barrier()

    # ---- stage 3: untied + pointwise + gate convs, batch-major ----
    # stationary = patch columns h_shift[:, px, py+ky, b-half] (81 x 128, FWL-
    # friendly), moving = per-position weights (81 x 65).  Output psum is
    # [128 b, 65] directly in batch-major layout: no transposes, no ACT copies.
    flatT = [flatpool.tile([BH, 800], F32, tag=f"flatT{h}", name=f"flatT{h}") for h in range(2)]

    for quad in range(25):
        pprime = (quad // 5) * 5 + (quad % 5)   # pooled position index py'*5+px'
        psq = [ps_q.tile([128, 4, 66], F32, tag="psq", name=f"psq{quad}_{i}") for i in range(2)]
        for qpos in range(4):
            qi = quad * 4 + qpos
            ci = qi // SCHUNK
            if qi % SCHUNK == 0 and ci + 1 not in s_chunks and ci + 1 < NCHUNKS:
                s_chunks[ci + 1] = load_chunk(ci + 1)
            s_t = s_chunks[ci]
            py, px = QUAD_ORDER[qi]
            for h in range(2):
                for ky in range(5):
                    nc.tensor.matmul(psq[h][:, qpos, 0:65],
                                     h_shift[:, px, py + ky, h * BH:(h + 1) * BH],
                                     s_t[:, qi % SCHUNK, ky, :],
                                     start=(ky == 0), stop=(ky == 4))
        for h in range(2):
            gsrc = sqpool.tile([128, 4, 1], F32, tag="gsrc", name=f"gsrc{quad}_{h}")
            nc.vector.tensor_copy(gsrc[:], psq[h][:, :, 64:65])
            sq = sqpool.tile([128, 4, 1], F32, tag="sq", name=f"sq{quad}_{h}")
            nc.scalar.activation(sq[:], gsrc[:], AF.Sigmoid)
            tmpq = mixpool.tile([128, 4, 32], F32, tag="tmpq")
            nc.vector.tensor_tensor(tmpq[:], psq[h][:, :, 0:32],
                                    sq[:].broadcast_to((128, 4, 32)), op=ALU.mult)
            mixq = mixpool.tile([128, 4, 32], F32, tag="mixq")
            nc.vector.tensor_tensor(mixq[:], tmpq[:], psq[h][:, :, 32:64], op=ALU.add)
            t1 = mixpool.tile([128, 2, 32], F32, tag="t1")
            nc.vector.tensor_tensor(t1[:], mixq[:, 0::2, :], mixq[:, 1::2, :], op=ALU.max)
            nc.vector.scalar_tensor_tensor(
                flatT[h][:, pprime::25], t1[:, 0, :], 0.0, t1[:, 1, :],
                op0=ALU.max, op1=ALU.max)

    # ---- stage 4: flat output DMA + re-transpose to K-major + fc ----
    for h in range(2):
        nc.sync.dma_start(O["flat"][h * BH:(h + 1) * BH, :], flatT[h][:])

    flatC = fcpool.tile([128, 7, B0], BF16, tag="flatC")
    nc.gpsimd.memset(flatC[32:64, 6, :], 1.0)  # row 32 = fc bias ones; 33+ never read (K=33)
    for j in range(7):
        w = 128 if j < 6 else 32
        for h in range(2):
            pst = ps_q.tile([128, 4, 66], F32, tag="psq", name=f"pst{j}_{h}")
            pt = pst[:].rearrange("p a b -> p (a b)")[0:w, 0:BH]
            nc.tensor.transpose(pt, flatT[h][:, j * 128:j * 128 + w],
                                identf[0:BH, 0:BH])
            nc.scalar.copy(flatC[0:w, j, h * BH:(h + 1) * BH], pt)

    psfc = ps_q.tile([128, 4, 66], F32, tag="psq", name="psfc")
    psfc_v = psfc[:].rearrange("p a b -> p (a b)")[0:10, 0:B0]
    for j in range(7):
        k = 128 if j < 6 else 33
        nc.tensor.matmul(psfc_v, fcwt[0:k, j, :], flatC[0:k, j, :],
                         start=(j == 0), stop=(j == 6))
    osb = fcpool.tile([10, B0], F32, tag="osb")
    nc.scalar.copy(osb[:], psfc_v)
    for h in range(2):
        pso2 = ps_q.tile([128, 4, 66], F32, tag="psq", name=f"pso2_{h}")
        po = pso2[:].rearrange("p a b -> p (a b)")[0:BH, 0:10]
        nc.tensor.transpose(po, osb[:, h * BH:(h + 1) * BH], identf[0:10, 0:10])
        ot = fcpool.tile([BH, 10], F32, tag=f"ot{h}", name=f"ot{h}")
        nc.scalar.copy(ot[:], po)
        nc.sync.dma_start(O["out"][h * BH:(h + 1) * BH, :], ot[:])


# ---------------------------------------------------------------------------
# host-side data prep
# ---------------------------------------------------------------------------

def _prep_weights(conv1_w, conv1_b, uc_w, uc_b, pc_w, pc_b, wl_w, wl_b, fc_w, fc_b):
    w1 = conv1_w[:, 0]  # [16, 5, 5]
    # M col layout: member*56 + co_l*14 + oxh; parity = ox % 2 selects the psum tile
    SA = np.zeros((2, 2, 128, 112), np.float32)
    SB = np.zeros((2, 2, 33, 112), np.float32)
    oxh = np.arange(14)
    for cp in range(2):
        for member in range(2):
            for co_l in range(4):
                co = cp * 8 + member * 4 + co_l
                col = member * 56 + co_l * 14 + oxh
                for par in range(2):
                    ox = 2 * oxh + par
                    for kx in range(5):
                        for g in range(4):
                            SA[cp, par, g * 32 + ox + kx, col] = w1[co, g, kx]
                        SB[cp, par, ox + kx, col] = w1[co, 4, kx]
                    SB[cp, par, 32, col] = conv1_b[co]

    # untied stationary: [81, 100(quad order), 5ky, 65]
    wu = uc_w.reshape(100, 16, 5, 5, 32)          # [p, c, ky, kx, o]
    wpc = pc_w.transpose(1, 2, 3, 0)               # [c, ky, kx, o]
    wwl = wl_w[0]                                  # [c, ky, kx]
    SU = np.zeros((81, 100, 5, 65), np.float32)
    for qi, (py, px) in enumerate(QUAD_ORDER):
        p = py * 10 + px
        for kx in range(5):
            r = slice(kx * 16, kx * 16 + 16)
            # SU[kx*16+c, qi, ky, :] — slices below are [c, ky, o] / [c, ky]
            SU[r, qi, :, 0:32] = wu[p, :, :, kx, :] - wpc[:, :, kx, :]
            SU[r, qi, :, 32:64] = wpc[:, :, kx, :]
            SU[r, qi, :, 64] = wwl[:, :, kx]
        SU[80, qi, 0, 0:32] = uc_b[0, :, py, px] - pc_b
        SU[80, qi, 0, 32:64] = pc_b
        SU[80, qi, 0, 64] = wl_b[0]

    fcwt = np.zeros((128, 7, 10), np.float32)
    for j in range(6):
        fcwt[:, j, :] = fc_w[:, j * 128:(j + 1) * 128].T
    fcwt[0:32, 6, :] = fc_w[:, 768:800].T
    fcwt[32, 6, :] = fc_b
    return (SA.astype(NPBF16), SB.astype(NPBF16), SU.astype(NPBF16),
            fcwt.astype(NPBF16))


def _prep_x(xs):
    """xs: [B0, 28, 28] f32 -> xb4 [128, 28, B0], xb1 [33, 28, B0] (bf16)."""
    xt = np.ascontiguousarray(xs.transpose(2, 1, 0))  # [xcol, yrow, b]
    xb4 = np.zeros((4, 32, 28, B0), np.float32)
    for g in range(4):
        ylo, yhi = max(0, 2 - g), min(28, 30 - g)
        xb4[g, 2:30, ylo:yhi, :] = xt[:, ylo + g - 2:yhi + g - 2, :]
    xb1 = np.zeros((33, 28, B0), np.float32)
    xb1[2:30, 0:26, :] = xt[:, 2:28, :]
    xb1[32] = 1.0
    return xb4.reshape(128, 28, B0).astype(NPBF16), xb1.astype(NPBF16)


_CACHED_NC = None


def _get_nc():
    global _CACHED_NC
    if _CACHED_NC is None:
        _CACHED_NC = build_program()
    return _CACHED_NC


def run(inputs, trace=False, **kw):
    x = np.asarray(inputs["x"], np.float32)
    SA, SB, SU, fcwt = _prep_weights(
        np.asarray(inputs["conv1_w"], np.float32), np.asarray(inputs["conv1_b"], np.float32),
        np.asarray(inputs["uc_w"], np.float32), np.asarray(inputs["uc_b"], np.float32),
        np.asarray(inputs["pc_w"], np.float32), np.asarray(inputs["pc_b"], np.float32),
        np.asarray(inputs["wl_w"], np.float32), np.asarray(inputs["wl_b"], np.float32),
        np.asarray(inputs["fc_w"], np.float32), np.asarray(inputs["fc_b"], np.float32))
    identb = np.eye(128, dtype=NPBF16)
    identf = np.eye(128, dtype=np.float32)

    in_maps = []
    for c in range(N_CORES):
        xb4, xb1 = _prep_x(x[c * B0:(c + 1) * B0, 0])
        in_maps.append({
            "xb4": xb4, "xb1": xb1, "sa": SA, "sb": SB, "su": SU,
            "fcwt": fcwt, "identb": identb, "identf": identf,
        })

    nc = _get_nc()
    res = run_bass_kernel_spmd(nc, in_maps, list(range(N_CORES)), trace=trace, **kw)
    flat = np.concatenate([res.results[c]["flat"] for c in range(N_CORES)], axis=0)
    out = np.concatenate([res.results[c]["out"] for c in range(N_CORES)], axis=0)
    return (out.astype(np.float32), flat.astype(np.float32)), res


def kernel(**inputs):
    (out, flat), _ = run(inputs)
    return (out, flat)


# revision 29
# speedup vs baseline: 1.6319x; 1.3411x over previous
"""Trainium2 Bass kernel for the dense CNN (conv1+pool -> untied/pointwise/gated mix -> pool -> fc).

Self-contained: hardcodes shapes for B=2048, 8-core data-parallel sharding.
kernel(**inputs) takes FULL inputs, returns (output [2048,10], flat [2048,800]).

Design (per core, 256 samples):
  stage 1  conv1 (1->16ch 5x5 pad2) as Toeplitz row-band matmuls: K=128 (4 y-shifted
           replicas x 32 padded cols) + K=33 bias pass; M=112 packs 8 channels x 14
           pooled-x columns, with even/odd conv-x in separate PSUM tiles so the 2x2
           maxpool is one free-dim tensor_reduce per parity + one fused max/relu.
  stage 2  SBUF->SBUF DMA shuffle into h_shift[(kx,c), px, y', b]: 5 x-shifted
           replicas of pooled h enabling zero-copy patch streaming (row 80 = ones).
  stage 3  untied + pointwise + gate convs fused: per output position, 5 accumulating
           matmuls (K=81) with stationary [d=wu-wpc | wpc | wl] (M=65, bias via ones
           row); PE-transpose to batch-major; sigmoid-gated mix at 128 partitions;
           2x2 maxpool fused into strided writes of flat.
  stage 4  fc: 7 accumulating matmuls over K=800 (+ones bias row), transpose, DMA out.
"""

import numpy as np
import ml_dtypes
from contextlib import ExitStack

import concourse.bass as bass
import concourse.tile as tile
from concourse import bacc
from concourse import mybir
from concourse.bass_utils import run_bass_kernel_spmd

F32 = mybir.dt.float32
BF16 = mybir.dt.bfloat16
NPBF16 = ml_dtypes.bfloat16

N_CORES = 8
B = 2048
B0 = B // N_CORES          # 256 samples per core
BH = B0 // 2               # 128, batch half (partition dim for transposed stages)

# quad-major position ordering for the 10x10 untied-conv grid (2x2 pool groups)
QUAD_ORDER = [(2 * qy + dy, 2 * qx + dx)
              for qy in range(5) for qx in range(5)
              for dy in range(2) for dx in range(2)]

SCHUNK = 20   # untied stationary streaming chunk (positions per DMA)
NCHUNKS = 100 // SCHUNK


def build_program():
    nc = bacc.Bacc("TRN2", target_bir_lowering=False, debug=False, num_devices=N_CORES)
    I = {}
    for name, shape, dt in [
        ("xb4", [128, 28, B0], BF16),
        ("xb1", [33, 28, B0], BF16),
        ("sa", [2, 2, 128, 112], BF16),     # [cp, parity, K, M]
        ("sb", [2, 2, 33, 112], BF16),
        ("su", [81, 100, 5, 65], BF16),
        ("fcwt", [128, 7, 10], BF16),
        ("ones1", [1, 10, 14, B0], BF16),
        ("identf", [128, 128], F32),
    ]:
        I[name] = nc.declare_dram_parameter(name, shape, dt, isOutput=False)
    O = {
        "flat": nc.declare_dram_parameter("flat", [B0, 800], F32, isOutput=True),
        "out": nc.declare_dram_parameter("out", [B0, 10], F32, isOutput=True),
    }

    with tile.TileContext(nc) as tc:
        with ExitStack() as ctx:
            _build(ctx, tc, I, O)
    nc.compile()
    return nc


def _build(ctx, tc, I, O):
    nc = tc.nc
    AF = mybir.ActivationFunctionType
    ALU = mybir.AluOpType
    AX = mybir.AxisListType

    consts = ctx.enter_context(tc.tile_pool(name="consts", bufs=1))
    spool = ctx.enter_context(tc.tile_pool(name="sstream", bufs=3))
    qpool = ctx.enter_context(tc.tile_pool(name="q", bufs=3))
    h2pool = ctx.enter_context(tc.tile_pool(name="h2", bufs=1))
    hspool = ctx.enter_context(tc.tile_pool(name="hshift", bufs=1))
    sqpool = ctx.enter_context(tc.tile_pool(name="sq", bufs=50))
    mixpool = ctx.enter_context(tc.tile_pool(name="mix", bufs=4))
    flatpool = ctx.enter_context(tc.tile_pool(name="flatT", bufs=1))
    fcpool = ctx.enter_context(tc.tile_pool(name="fcmisc", bufs=1))

    ps_c1 = ctx.enter_context(tc.tile_pool(name="ps_c1", bufs=4, space="PSUM"))
    ps_q = ctx.enter_context(tc.tile_pool(name="ps_q", bufs=4, space="PSUM"))

    # ---- load persistent constants ----
    xb4 = consts.tile([128, 28, B0], BF16)
    nc.sync.dma_start(xb4[:], I["xb4"][:])
    xb1 = consts.tile([33, 28, B0], BF16)
    nc.sync.dma_start(xb1[:], I["xb1"][:])
    sa_t = [[consts.tile([128, 112], BF16, tag=f"sa{cp}{par}", name=f"sa{cp}{par}")
             for par in range(2)] for cp in range(2)]
    sb_t = [[consts.tile([33, 112], BF16, tag=f"sb{cp}{par}", name=f"sb{cp}{par}")
             for par in range(2)] for cp in range(2)]
    for cp in range(2):
        for par in range(2):
            nc.sync.dma_start(sa_t[cp][par][:], I["sa"][cp, par])
            nc.sync.dma_start(sb_t[cp][par][:], I["sb"][cp, par])
    fcwt = consts.tile([128, 7, 10], BF16, tag="fcwt")
    nc.sync.dma_start(fcwt[:], I["fcwt"][:])
    identf = consts.tile([128, 128], F32, tag="identf")
    nc.sync.dma_start(identf[:], I["identf"][:])

    # h_shift[(kx*16+c), px, y', b] = pooled_h[c, y', px+kx, b];  row 80 = ones (bias row)
    h_shift = hspool.tile([81, 10, 14, B0], BF16)
    nc.scalar.dma_start(h_shift[80:81, :, :, :], I["ones1"][:])

    def load_chunk(ci):
        t = spool.tile([81, SCHUNK, 5, 65], BF16, tag="su", name=f"su{ci}")
        nc.sync.dma_start(t[:], I["su"][:, ci * SCHUNK:(ci + 1) * SCHUNK, :, :])
        return t
    s_chunks = {0: load_chunk(0), 1: load_chunk(1)}

    # internal DRAM staging for the shuffle (DRAM APs reshape freely, letting
    # 20 matched-4D DMAs replace 80 per-channel ones; dma_start issue overhead
    # is ~2.3us each, so instruction count dominates the shuffle cost)
    h2d = [nc.dram_tensor(f"h2d{cp}", [112, 14, B0], BF16) for cp in range(2)]

    # ---- stage 1+2: conv1 (Toeplitz row-band) + relu + 2x2 maxpool; shuffle
    # DMAs issued per cp so they overlap the other cp's convolutions ----
    # M layout: col = member*56 + co_l*14 + oxh;  channel c = cp*8 + member*4 + co_l
    dma_engs = [nc.sync, nc.scalar]   # the two HWDGE-capable issuers
    for cp in range(2):
        h2 = h2pool.tile([112, 14, B0], BF16, tag=f"h2_{cp}", name=f"h2_{cp}")
        for yp in range(14):
            y = yp * 2
            pse = ps_c1.tile([112, 2, B0], F32, tag="psc1", name=f"pse{cp}_{yp}")
            nc.tensor.matmul(pse[:], sa_t[cp][0][:], xb4[:, y:y + 2, :],
                             start=True, stop=False)
            nc.tensor.matmul(pse[:], sb_t[cp][0][:], xb1[:, y:y + 2, :],
                             start=False, stop=True)
            pso = ps_c1.tile([112, 2, B0], F32, tag="psc1", name=f"pso{cp}_{yp}")
            nc.tensor.matmul(pso[:], sa_t[cp][1][:], xb4[:, y:y + 2, :],
                             start=True, stop=False)
            nc.tensor.matmul(pso[:], sb_t[cp][1][:], xb1[:, y:y + 2, :],
                             start=False, stop=True)
            # maxpool over conv-y pair (innermost after free permute), per parity
            q1 = qpool.tile([112, B0], BF16, tag="q1")
            nc.vector.tensor_reduce(q1[:], pse[:].rearrange("p a b -> p b a"),
                                    axis=AX.X, op=ALU.max)
            q2 = qpool.tile([112, B0], BF16, tag="q2")
            nc.vector.tensor_reduce(q2[:], pso[:].rearrange("p a b -> p b a"),
                                    axis=AX.X, op=ALU.max)
            # maxpool over conv-x parity + relu: h2 = max(max(q1, 0), q2)
            nc.vector.scalar_tensor_tensor(h2[:, yp, :], q1[:], 0.0, q2[:],
                                           op0=ALU.max, op1=ALU.max)
        # stage 2a: dump pooled h to DRAM (one contiguous DMA per cp)
        dma_engs[cp].dma_start(h2d[cp][:], h2[:])
        # stage 2b: scatter into h_shift; DRAM src reshaped [m, co_l, oxh, y, b]
        h2v = h2d[cp].rearrange("(m c o) y b -> m c o y b", m=2, c=4)
        for kx in range(5):
            for member in range(2):
                c0 = cp * 8 + member * 4
                src = h2v[member, :, kx:kx + 10, :, :]                 # [4, 10, 14, B0]
                dst = h_shift[kx * 16 + c0: kx * 16 + c0 + 4, :, :, :]  # [4, 10, 14, B0]
                dma_engs[(kx + member + cp) % 2].dma_start(dst, src)

    # ---- stage 3: untied + pointwise + gate convs, batch-major ----
    # stationary = patch columns h_shift[:, px, py+ky, b-half] (81 x 128, FWL-
    # friendly), moving = per-position weights (81 x 65).  Output psum is
    # [128 b, 65] directly in batch-major layout: no transposes, no ACT copies.
    flatT = [flatpool.tile([BH, 800], F32, tag=f"flatT{h}", name=f"flatT{h}") for h in range(2)]

    for quad in range(25):
        pprime = (quad // 5) * 5 + (quad % 5)   # pooled position index py'*5+px'
        psq = [ps_q.tile([128, 4, 66], F32, tag="psq", name=f"psq{quad}_{i}") for i in range(2)]
        for qpos in range(4):
            qi = quad * 4 + qpos
            ci = qi // SCHUNK
            if qi % SCHUNK == 0 and ci + 1 not in s_chunks and ci + 1 < NCHUNKS:
                s_chunks[ci + 1] = load_chunk(ci + 1)
            s_t = s_chunks[ci]
            py, px = QUAD_ORDER[qi]
            for h in range(2):
                for ky in range(5):
                    nc.tensor.matmul(psq[h][:, qpos, 0:65],
                                     h_shift[:, px, py + ky, h * BH:(h + 1) * BH],
                                     s_t[:, qi % SCHUNK, ky, :],
                                     start=(ky == 0), stop=(ky == 4))
        for h in range(2):
            sq = sqpool.tile([128, 4, 1], F32, tag="sq", name=f"sq{quad}_{h}")
            nc.scalar.activation(sq[:], psq[h][:, :, 64:65], AF.Sigmoid)
            # d * sigmoid(g) on the scalar engine (per-partition scale AP)
            tmpq = mixpool.tile([128, 4, 32], F32, tag="tmpq")
            for qpos in range(4):
                nc.scalar.activation(tmpq[:, qpos, :], psq[h][:, qpos, 0:32],
                                     AF.Copy, bias=0.0, scale=sq[:, qpos, :])
            mixq = mixpool.tile([128, 4, 32], F32, tag="mixq")
            nc.vector.tensor_tensor(mixq[:], tmpq[:], psq[h][:, :, 32:64], op=ALU.add)
            t1 = mixpool.tile([128, 2, 32], F32, tag="t1")
            nc.vector.tensor_tensor(t1[:], mixq[:, 0::2, :], mixq[:, 1::2, :], op=ALU.max)
            nc.vector.scalar_tensor_tensor(
                flatT[h][:, pprime::25], t1[:, 0, :], 0.0, t1[:, 1, :],
                op0=ALU.max, op1=ALU.max)

    # ---- stage 4: flat output DMA + re-transpose to K-major + fc ----
    for h in range(2):
        nc.sync.dma_start(O["flat"][h * BH:(h + 1) * BH, :], flatT[h][:])

    flatC = fcpool.tile([128, 7, B0], BF16, tag="flatC")
    nc.gpsimd.memset(flatC[32:64, 6, :], 1.0)  # row 32 = fc bias ones; 33+ never read (K=33)
    for j in range(7):
        w = 128 if j < 6 else 32
        for h in range(2):
            pst = ps_q.tile([128, 4, 66], F32, tag="psq", name=f"pst{j}_{h}")
            pt = pst[:].rearrange("p a b -> p (a b)")[0:w, 0:BH]
            nc.tensor.transpose(pt, flatT[h][:, j * 128:j * 128 + w],
                                identf[0:BH, 0:BH])
            nc.scalar.copy(flatC[0:w, j, h * BH:(h + 1) * BH], pt)

    psfc = ps_q.tile([128, 4, 66], F32, tag="psq", name="psfc")
    psfc_v = psfc[:].rearrange("p a b -> p (a b)")[0:10, 0:B0]
    for j in range(7):
        k = 128 if j < 6 else 33
        nc.tensor.matmul(psfc_v, fcwt[0:k, j, :], flatC[0:k, j, :],
                         start=(j == 0), stop=(j == 6))
    osb = fcpool.tile([10, B0], F32, tag="osb")
    nc.scalar.copy(osb[:], psfc_v)
    for h in range(2):
        pso2 = ps_q.tile([128, 4, 66], F32, tag="psq", name=f"pso2_{h}")
        po = pso2[:].rearrange("p a b -> p (a b)")[0:BH, 0:10]
        nc.tensor.transpose(po, osb[:, h * BH:(h + 1) * BH], identf[0:10, 0:10])
        ot = fcpool.tile([BH, 10], F32, tag=f"ot{h}", name=f"ot{h}")
        nc.scalar.copy(ot[:], po)
        nc.sync.dma_start(O["out"][h * BH:(h + 1) * BH, :], ot[:])


# ---------------------------------------------------------------------------
# host-side data prep
# ---------------------------------------------------------------------------

def _prep_weights(conv1_w, conv1_b, uc_w, uc_b, pc_w, pc_b, wl_w, wl_b, fc_w, fc_b):
    w1 = conv1_w[:, 0]  # [16, 5, 5]
    # M col layout: member*56 + co_l*14 + oxh; parity = ox % 2 selects the psum tile
    SA = np.zeros((2, 2, 128, 112), np.float32)
    SB = np.zeros((2, 2, 33, 112), np.float32)
    oxh = np.arange(14)
    for cp in range(2):
        for member in range(2):
            for co_l in range(4):
                co = cp * 8 + member * 4 + co_l
                col = member * 56 + co_l * 14 + oxh
                for par in range(2):
                    ox = 2 * oxh + par
                    for kx in range(5):
                        for g in range(4):
                            SA[cp, par, g * 32 + ox + kx, col] = w1[co, g, kx]
                        SB[cp, par, ox + kx, col] = w1[co, 4, kx]
                    SB[cp, par, 32, col] = conv1_b[co]

    # untied stationary: [81, 100(quad order), 5ky, 65]
    wu = uc_w.reshape(100, 16, 5, 5, 32)          # [p, c, ky, kx, o]
    wpc = pc_w.transpose(1, 2, 3, 0)               # [c, ky, kx, o]
    wwl = wl_w[0]                                  # [c, ky, kx]
    SU = np.zeros((81, 100, 5, 65), np.float32)
    for qi, (py, px) in enumerate(QUAD_ORDER):
        p = py * 10 + px
        for kx in range(5):
            r = slice(kx * 16, kx * 16 + 16)
            # SU[kx*16+c, qi, ky, :] — slices below are [c, ky, o] / [c, ky]
            SU[r, qi, :, 0:32] = wu[p, :, :, kx, :] - wpc[:, :, kx, :]
            SU[r, qi, :, 32:64] = wpc[:, :, kx, :]
            SU[r, qi, :, 64] = wwl[:, :, kx]
        SU[80, qi, 0, 0:32] = uc_b[0, :, py, px] - pc_b
        SU[80, qi, 0, 32:64] = pc_b
        SU[80, qi, 0, 64] = wl_b[0]

    fcwt = np.zeros((128, 7, 10), np.float32)
    for j in range(6):
        fcwt[:, j, :] = fc_w[:, j * 128:(j + 1) * 128].T
    fcwt[0:32, 6, :] = fc_w[:, 768:800].T
    fcwt[32, 6, :] = fc_b
    return (SA.astype(NPBF16), SB.astype(NPBF16), SU.astype(NPBF16),
            fcwt.astype(NPBF16))


def _prep_x(xs):
    """xs: [B0, 28, 28] f32 -> xb4 [128, 28, B0], xb1 [33, 28, B0] (bf16)."""
    xt = np.ascontiguousarray(xs.transpose(2, 1, 0))  # [xcol, yrow, b]
    xb4 = np.zeros((4, 32, 28, B0), np.float32)
    for g in range(4):
        ylo, yhi = max(0, 2 - g), min(28, 30 - g)
        xb4[g, 2:30, ylo:yhi, :] = xt[:, ylo + g - 2:yhi + g - 2, :]
    xb1 = np.zeros((33, 28, B0), np.float32)
    xb1[2:30, 0:26, :] = xt[:, 2:28, :]
    xb1[32] = 1.0
    return xb4.reshape(128, 28, B0).astype(NPBF16), xb1.astype(NPBF16)


_CACHED_NC = None


def _get_nc():
    global _CACHED_NC
    if _CACHED_NC is None:
        _CACHED_NC = build_program()
    return _CACHED_NC


def run(inputs, trace=False, **kw):
    x = np.asarray(inputs["x"], np.float32)
    SA, SB, SU, fcwt = _prep_weights(
        np.asarray(inputs["conv1_w"], np.float32), np.asarray(inputs["conv1_b"], np.float32),
        np.asarray(inputs["uc_w"], np.float32), np.asarray(inputs["uc_b"], np.float32),
        np.asarray(inputs["pc_w"], np.float32), np.asarray(inputs["pc_b"], np.float32),
        np.asarray(inputs["wl_w"], np.float32), np.asarray(inputs["wl_b"], np.float32),
        np.asarray(inputs["fc_w"], np.float32), np.asarray(inputs["fc_b"], np.float32))
    ones1 = np.ones((1, 10, 14, B0), NPBF16)
    identf = np.eye(128, dtype=np.float32)

    in_maps = []
    for c in range(N_CORES):
        xb4, xb1 = _prep_x(x[c * B0:(c + 1) * B0, 0])
        in_maps.append({
            "xb4": xb4, "xb1": xb1, "sa": SA, "sb": SB, "su": SU,
            "fcwt": fcwt, "ones1": ones1, "identf": identf,
        })

    nc = _get_nc()
    res = run_bass_kernel_spmd(nc, in_maps, list(range(N_CORES)), trace=trace, **kw)
    flat = np.concatenate([res.results[c]["flat"] for c in range(N_CORES)], axis=0)
    out = np.concatenate([res.results[c]["out"] for c in range(N_CORES)], axis=0)
    return (out.astype(np.float32), flat.astype(np.float32)), res


def kernel(**inputs):
    (out, flat), _ = run(inputs)
    return (out, flat)
